# revision 36
# baseline (speedup 1.0000x reference)
# Trainium2 Bass kernel for DoubleXLSTMUp (2x mLSTM blocks + in/out proj).
#
# Sharding: 8 cores = 4 batches x 2 head-groups. Each core computes one
# batch's full sequence with 2 of the 4 mLSTM heads; the inner-dim-split
# down-projection partial sums are AllReduced across each core pair.
# The mLSTM attention uses the chunkwise-parallel stabilized form
# (running state S[dh,dh], z[dh], running max) with chunk length 128.
# v is computed as xn @ (W_up_xin @ Wv) -- the weight product is folded
# on the host, which removes the x_in dependency for v.
# The residual stream x, z, and h are staged in DRAM to fit SBUF.
import numpy as np
import ml_dtypes
from contextlib import ExitStack

import concourse.bass as bass
import concourse.bacc as bacc
import concourse.mybir as mybir
import concourse.tile as tile
from concourse.bass import ts, ds
from concourse.bass_utils import run_bass_kernel_spmd
from concourse.masks import make_identity, make_upper_triangular

F32 = mybir.dt.float32
BF16 = mybir.dt.bfloat16
AF = mybir.ActivationFunctionType
OP = mybir.AluOpType

EMB, INIT, INNER, H, DH, KCONV = 512, 256, 1024, 4, 256, 4
B, S = 4, 2048
NCORES = 8
HPC = 2            # heads per core
CIN = HPC * DH     # 512: my half of INNER
L = 128            # mLSTM chunk length
NCH = S // L       # 16 chunks
TT = S // 128      # 16 token tiles
P = 128
EPS = 1e-6
RG = [[0, 1], [2, 3], [4, 5], [6, 7]]  # core pairs sharing a batch

# Engine SBUF constraints: accesses must start at a 32-aligned partition,
# and two-SBUF-input ops need EQUAL base partitions.  So every per-head
# [2, S] row quantity lives at partition base 0, packed along the free dim
# of one [32, 3, S] tile; the three blocks are aggressively aliased.
C_CM, C_MI, C_ME, C_SF = 0, 1, 2, 3   # blocks of the [32, 4, NCH] stat tile


def _qs(rowq, blk):
    return rowq[0:HPC, blk, :]


def _cs(cbq, blk):
    return cbq[0:HPC, blk, :]

_cached = {}


def _decl_inputs(nc):
    t = {}
    t["x1T"] = nc.dram_tensor("x1T", [INIT, S], BF16, kind="ExternalInput")
    t["x2"] = nc.dram_tensor("x2", [S, EMB], F32, kind="ExternalInput")
    t["w_init"] = nc.dram_tensor("w_init", [INIT, EMB], BF16, kind="ExternalInput")
    for kb in (0, 1):
        sf = f"_{kb}"
        t["ln_g" + sf] = nc.dram_tensor("ln_g" + sf, [1, EMB], F32, kind="ExternalInput")
        t["ln_b" + sf] = nc.dram_tensor("ln_b" + sf, [1, EMB], F32, kind="ExternalInput")
        t["w_up" + sf] = nc.dram_tensor("w_up" + sf, [EMB, INNER + CIN], BF16, kind="ExternalInput")
        t["wv" + sf] = nc.dram_tensor("wv" + sf, [EMB, CIN], BF16, kind="ExternalInput")
        t["conv_w" + sf] = nc.dram_tensor("conv_w" + sf, [INNER, KCONV], F32, kind="ExternalInput")
        t["conv_b" + sf] = nc.dram_tensor("conv_b" + sf, [INNER], F32, kind="ExternalInput")
        t["wq" + sf] = nc.dram_tensor("wq" + sf, [INNER, CIN], BF16, kind="ExternalInput")
        t["wk" + sf] = nc.dram_tensor("wk" + sf, [INNER, CIN], BF16, kind="ExternalInput")
        t["wif" + sf] = nc.dram_tensor("wif" + sf, [INNER, 2 * HPC], BF16, kind="ExternalInput")
        t["bif" + sf] = nc.dram_tensor("bif" + sf, [2 * HPC, 1], F32, kind="ExternalInput")
        t["skip" + sf] = nc.dram_tensor("skip" + sf, [1, CIN], F32, kind="ExternalInput")
        t["gn_g" + sf] = nc.dram_tensor("gn_g" + sf, [1, CIN], F32, kind="ExternalInput")
        t["gn_b" + sf] = nc.dram_tensor("gn_b" + sf, [1, CIN], F32, kind="ExternalInput")
        t["w_down" + sf] = nc.dram_tensor("w_down" + sf, [CIN, EMB], BF16, kind="ExternalInput")
    t["ln_out_g"] = nc.dram_tensor("ln_out_g", [1, EMB], F32, kind="ExternalInput")
    t["ln_out_b"] = nc.dram_tensor("ln_out_b", [1, EMB], F32, kind="ExternalInput")
    return t


def _load_w(nc, pool, dram, name):
    """Load a [E, N] weight into sbuf [128, E//128, N] (contraction on partitions)."""
    e, n = dram.shape
    ko = e // P
    sb = pool.tile([P, ko, n], dram.dtype, name=name)
    nc.sync.dma_start(out=sb, in_=dram[:].rearrange("(ko p) n -> p ko n", p=P))
    return sb


def _bcast(nc, pool, pools, dram_row, name):
    """[1, n] dram row -> [128, n] sbuf, replicated across partitions via a
    stride-0 DMA read."""
    n = dram_row.shape[-1]
    out = pool.tile([P, n], dram_row.dtype, name=name)
    nc.sync.dma_start(out=out, in_=dram_row[0:1, :].to_broadcast((P, n)))
    return out


def _ln_tile(nc, pools, xt, g_bc, b_bc, out_ap):
    """One [128, EMB] layernorm: out = (xt - mu) * rstd * g + b."""
    small = pools["small"]
    stats = small.tile([P, 6], F32, tag="ln_stats")
    nc.vector.bn_stats(out=stats, in_=xt)
    mv = small.tile([P, 2], F32, tag="ln_mv")
    nc.vector.bn_aggr(out=mv, in_=stats)
    std = small.tile([P, 1], F32, tag="ln_std")
    nc.scalar.activation(out=std, in_=mv[:, 1:2], func=AF.Sqrt, bias=pools["eps"])
    rstd = small.tile([P, 1], F32, tag="ln_rstd")
    nc.vector.reciprocal(out=rstd, in_=std)
    t0 = small.tile([P, EMB], F32, tag="ln_t0", bufs=2)
    nc.vector.tensor_scalar(out=t0, in0=xt, scalar1=mv[:, 0:1], scalar2=rstd,
                            op0=OP.subtract, op1=OP.mult)
    nc.vector.tensor_mul(t0, t0, g_bc)
    nc.any.tensor_add(out_ap, t0, b_bc)


def _mlstm_block(nc, ctx, tc, pools, t, kb, xdr, dbg):
    """One mLSTM block; updates the DRAM residual stream xdr in place.

    Tile pools are a stack allocator (LIFO frees), so pools are opened in
    decreasing order of lifetime.
    """
    sf = f"_{kb}"
    ident = pools["ident"]
    trimask = pools["trimask"]
    ones_col = pools["ones_col"]
    dram = pools["dram"]
    small = pools["small"]

    z_d = dram.tile([S, CIN], BF16, name=f"z_d{kb}")
    z_v = z_d[:].rearrange("(n p) c -> p n c", p=P)
    h_d = dram.tile([S, CIN], BF16, name=f"h_d{kb}")
    h_v = h_d[:].rearrange("(n p) c -> p n c", p=P)

    # --- block-lifetime pools (opened first; closed last) ---
    wrest = ctx.enter_context(tc.tile_pool(name=f"wrest{kb}", bufs=1))
    wif = _load_w(nc, wrest, t["wif" + sf], "wif_sb")             # [128,8,4]
    w_down = _load_w(nc, wrest, t["w_down" + sf], "w_down_sb")    # [128,4,512]
    convw = _load_w(nc, wrest, t["conv_w" + sf], "convw_sb")      # [128,8,4]
    convb_t = wrest.tile([P, INNER // P], F32, name="convb_sb")   # [128,8]
    nc.sync.dma_start(out=convb_t, in_=t["conv_b" + sf][:].rearrange("(ko p) -> p ko", p=P))
    bi_t = wrest.tile([HPC, 1], F32, name="bi_sb")
    nc.sync.dma_start(out=bi_t, in_=t["bif" + sf][0:HPC, :])
    bf_t = wrest.tile([HPC, 1], F32, name="bf_sb")
    nc.sync.dma_start(out=bf_t, in_=t["bif" + sf][HPC:2 * HPC, :])
    skip_bc = _bcast(nc, wrest, pools, t["skip" + sf], "skip_bc")
    gn_g_bc = _bcast(nc, wrest, pools, t["gn_g" + sf], "gn_g_bc")
    gn_b_bc = _bcast(nc, wrest, pools, t["gn_b" + sf], "gn_b_bc")

    xc_d = dram.tile([S, CIN], BF16, name=f"xc_d{kb}")
    xc_v = xc_d[:].rearrange("(n p) c -> p n c", p=P)
    cols = ctx.enter_context(tc.tile_pool(name=f"cols{kb}", bufs=1))
    rowq = cols.tile([32, 3, S], F32, name="rowq")
    cbq = cols.tile([32, 4, NCH], F32, name="cbq")
    zeroP = cols.tile([32, 2], F32, name="zeroP")
    ecol = cols.tile([P, HPC, NCH], F32, name="ecol")
    egcol = cols.tile([P, HPC, NCH], F32, name="egcol")
    ekcol_f = cols.tile([P, HPC, NCH], F32, name="ekcol_f")
    ekcol_b = cols.tile([P, HPC, NCH], BF16, name="ekcol_b")
    sfac_bc = cols.tile([P, HPC, NCH], F32, name="sfac_bc")
    rncol = cols.tile([P, HPC, NCH], F32, name="rncol")
    S_st = cols.tile([P, HPC, 2, DH], BF16, name="S_st")
    z_st = cols.tile([P, HPC, 2], BF16, name="z_st")

    # --- phases 1-6 under the vqk pool (v, qT, kT live until chunk loop) ---
    with tc.tile_pool(name=f"vqk{kb}", bufs=1) as vqk:
        v_tok = vqk.tile([P, TT, CIN], BF16, name="v_tok")
        qT = vqk.tile([P, CIN // P, S], BF16, name="qT")
        kT = vqk.tile([P, CIN // P, S], BF16, name="kT")

        with tc.tile_pool(name=f"xct{kb}", bufs=1) as xct_pool:
            xcT = xct_pool.tile([P, INNER // P, S], BF16, name="xcT")

            # phase 1-3: LN -> xnT (per-tile transpose); z, v; up-proj+conv
            with tc.tile_pool(name=f"xnT{kb}", bufs=1) as xnT_pool, \
                 tc.tile_pool(name=f"wup{kb}", bufs=1) as wup_pool, \
                 tc.tile_pool(name=f"ph2{kb}", bufs=1) as ph2, \
                 tc.tile_pool(name=f"ph12ps{kb}", bufs=1, space="PSUM") as ps:
                w_up = _load_w(nc, wup_pool, t["w_up" + sf], "w_up_sb")    # [128,4,1536]
                xnT = xnT_pool.tile([P, EMB // P, S], BF16, name="xnT")
                ln_g_bc = _bcast(nc, ph2, pools, t["ln_g" + sf], "ln_g_bc")
                ln_b_bc = _bcast(nc, ph2, pools, t["ln_b" + sf], "ln_b_bc")
                for tt_i in range(TT):
                    xt = small.tile([P, EMB], F32, tag="ln_x", bufs=3)
                    nc.sync.dma_start(out=xt, in_=xdr[:, tt_i, :])
                    xn_t = ph2.tile([P, EMB], BF16, tag="xn_t", bufs=3)
                    _ln_tile(nc, pools, xt, ln_g_bc, ln_b_bc, xn_t)
                    for ip in range(EMB // P):
                        pt = ps.tile([P, P], BF16, tag="tr_xn", bufs=2)
                        nc.tensor.transpose(pt, xn_t[:, ts(ip, P)], ident)
                        nc.any.tensor_copy(xnT[:, ip, ts(tt_i, P)], pt)
                # z and v (both per tt: lhsT=xnT slice, rhs [512,512])
                with tc.tile_pool(name=f"wv{kb}", bufs=1) as wv_pool:
                    wv_e = _load_w(nc, wv_pool, t["wv" + sf], "wv_sb")     # [128,4,512]
                    for tt_i in range(TT):
                        ptv = ps.tile([P, CIN], F32, tag="v_ps", bufs=2)
                        for k in range(EMB // P):
                            nc.tensor.matmul(ptv, lhsT=xnT[:, k, ts(tt_i, P)],
                                             rhs=wv_e[:, k, :],
                                             start=(k == 0), stop=(k == EMB // P - 1))
                        nc.any.tensor_copy(v_tok[:, tt_i, :], ptv)
                        ptz = ps.tile([P, CIN], F32, tag="z_ps", bufs=2)
                        for k in range(EMB // P):
                            nc.tensor.matmul(ptz, lhsT=xnT[:, k, ts(tt_i, P)],
                                             rhs=w_up[:, k, INNER:INNER + CIN],
                                             start=(k == 0), stop=(k == EMB // P - 1))
                        zt = ph2.tile([P, CIN], BF16, tag="z_t", bufs=3)
                        nc.any.tensor_copy(zt, ptz)
                        nc.sync.dma_start(out=z_v[:, tt_i, :], in_=zt)
                # up-projection + causal conv + silu, one 128-ch slab at a time
                for co in range(INNER // P):
                    xrow = ph2.tile([P, KCONV - 1 + S], BF16, tag="x_row", bufs=2)
                    nc.vector.memset(xrow[:, 0:KCONV - 1], 0.0)
                    for nt in range(S // 512):
                        pt = ps.tile([P, 512], F32, tag="up_ps", bufs=2)
                        for k in range(EMB // P):
                            nc.tensor.matmul(pt, lhsT=w_up[:, k, ts(co, P)],
                                             rhs=xnT[:, k, ts(nt, 512)],
                                             start=(k == 0), stop=(k == EMB // P - 1))
                        nc.any.tensor_copy(
                            xrow[:, KCONV - 1 + nt * 512:KCONV - 1 + (nt + 1) * 512], pt)
                    acc = ph2.tile([P, S], F32, tag="conv_acc", bufs=1)
                    nc.vector.tensor_scalar(
                        out=acc, in0=xrow[:, 0:S], scalar1=convw[:, co, 0:1],
                        scalar2=None, op0=OP.mult)
                    for k in range(1, KCONV):
                        nc.vector.scalar_tensor_tensor(
                            out=acc, in0=xrow[:, k:k + S], scalar=convw[:, co, k:k + 1],
                            in1=acc, op0=OP.mult, op1=OP.add)
                    nc.scalar.activation(out=xcT[:, co, :], in_=acc, func=AF.Silu,
                                         bias=convb_t[:, co:co + 1])

            # phase 4: q,k,if projections; xc_tok transpose
            with tc.tile_pool(name=f"wqk{kb}", bufs=1) as wqk_pool, \
                 tc.tile_pool(name=f"ph4ps{kb}", bufs=1, space="PSUM") as ps4:
                wq = _load_w(nc, wqk_pool, t["wq" + sf], "wq_sb")
                wk = _load_w(nc, wqk_pool, t["wk" + sf], "wk_sb")
                for (wt, dst, scale) in ((wq, qT, 1.0 / 16.0), (wk, kT, 1.0)):
                    for co in range(CIN // P):
                        for nt in range(S // 512):
                            pt = ps4.tile([P, 512], F32, tag="qk_ps", bufs=3)
                            for k in range(INNER // P):
                                nc.tensor.matmul(pt, lhsT=wt[:, k, ts(co, P)],
                                                 rhs=xcT[:, k, ts(nt, 512)],
                                                 start=(k == 0),
                                                 stop=(k == INNER // P - 1))
                            if scale == 1.0:
                                nc.any.tensor_copy(dst[:, co, ts(nt, 512)], pt)
                            else:
                                nc.scalar.activation(out=dst[:, co, ts(nt, 512)],
                                                     in_=pt, func=AF.Copy, scale=scale)
                for nt in range(S // 512):
                    for (c0, blk, bias_t) in ((0, 0, bi_t), (HPC, 1, bf_t)):
                        pt = ps4.tile([HPC, 512], F32, tag="if_ps", bufs=2)
                        for k in range(INNER // P):
                            nc.tensor.matmul(pt, lhsT=wif[:, k, c0:c0 + HPC],
                                             rhs=xcT[:, k, ts(nt, 512)],
                                             start=(k == 0), stop=(k == INNER // P - 1))
                        nc.scalar.activation(
                            out=_qs(rowq, blk)[:, ts(nt, 512)], in_=pt,
                            func=AF.Identity, bias=bias_t)
                for if_ in range(TT):
                    xct_t = small.tile([P, CIN], BF16, tag="xct_t", bufs=3)
                    for ip in range(CIN // P):
                        pt = ps4.tile([P, P], BF16, tag="tr_xc", bufs=3)
                        nc.tensor.transpose(pt, xcT[:, ip, ts(if_, P)], ident)
                        nc.any.tensor_copy(xct_t[:, ts(ip, P)], pt)
                    nc.sync.dma_start(out=xc_v[:, if_, :], in_=xct_t)

        # phases 5+6 (xcT freed)
        br_f = dram.tile([3, HPC, S], F32, name=f"br_f{kb}")
        rs_d = _phase56(nc, tc, pools, kb, rowq, cbq, zeroP, ecol, egcol,
                        ekcol_f, ekcol_b, sfac_bc, S_st, z_st, h_v, qT, kT,
                        v_tok, trimask, ones_col, ident, br_f, dbg)

    # ---- phase 7: normalizer n = max(|rs * er|, enm); rn = 1/n ----
    # b2 (g) is dead; load rs there.  b0 holds ek (dead); reload er into it.
    rs_r = _qs(rowq, 2)
    nc.sync.dma_start(out=rs_r, in_=rs_d[:])
    er_r = _qs(rowq, 0)
    nc.sync.dma_start(out=er_r, in_=br_f[0])
    nc.vector.tensor_mul(rs_r, rs_r, er_r)
    nc.scalar.activation(out=rs_r, in_=rs_r, func=AF.Abs)
    nc.vector.tensor_max(rs_r, rs_r, _qs(rowq, 1))
    nc.vector.reciprocal(out=rs_r, in_=rs_r)
    br_rn = pools["dram"].tile([HPC, S], F32, name=f"br_rn{kb}")
    nc.sync.dma_start(out=br_rn, in_=rs_r)
    for hh in range(HPC):
        nc.sync.dma_start(out=rncol[:, hh, :],
                          in_=br_rn[hh].rearrange("(c l) -> l c", l=L))

    if dbg is not None:
        nc.sync.dma_start(out=dbg[f"er{kb}"][:], in_=er_r)
        nc.sync.dma_start(out=dbg[f"enm{kb}"][:], in_=_qs(rowq, 1))
        nc.sync.dma_start(out=dbg[f"rn{kb}"][:], in_=rs_r)
        nc.sync.dma_start(out=dbg[f"h{kb}"][:], in_=h_d[:])

    # ---- phase 8: h/n, groupnorm, h_out, down-proj, AllReduce, residual ----
    par_d = dram.tile([S, EMB], BF16, name=f"par{kb}")
    ar_d = dram.tile([S, EMB], BF16, name=f"ar{kb}")
    par_v = par_d[:].rearrange("(n p) c -> p n c", p=P)
    ar_v = ar_d[:].rearrange("(n p) c -> p n c", p=P)
    with tc.tile_pool(name=f"ph8_{kb}", bufs=2) as ph8, \
         tc.tile_pool(name=f"ph8ps_{kb}", bufs=1, space="PSUM") as ps8:
        houtT = ph8.tile([P, CIN // P, S], BF16, name="houtT", bufs=1)
        for tt_i in range(TT):
            ht = ph8.tile([P, CIN], BF16, tag="h_in", bufs=3)
            nc.sync.dma_start(out=ht, in_=h_v[:, tt_i, :])
            hn = ph8.tile([P, CIN], F32, tag="hn")
            for hh in range(HPC):
                hsl = ht[:, hh * DH:(hh + 1) * DH]
                nc.vector.tensor_scalar(
                    out=hsl, in0=hsl, scalar1=rncol[:, hh, tt_i:tt_i + 1],
                    scalar2=None, op0=OP.mult)
                stats = small.tile([P, 6], F32, tag="gn_stats")
                nc.vector.bn_stats(out=stats, in_=hsl)
                mv = small.tile([P, 2], F32, tag="gn_mv")
                nc.vector.bn_aggr(out=mv, in_=stats)
                std = small.tile([P, 1], F32, tag="gn_std")
                nc.scalar.activation(out=std, in_=mv[:, 1:2], func=AF.Sqrt,
                                     bias=pools["eps"])
                rstd = small.tile([P, 1], F32, tag="gn_rstd")
                nc.vector.reciprocal(out=rstd, in_=std)
                nc.vector.tensor_scalar(
                    out=hn[:, hh * DH:(hh + 1) * DH], in0=hsl,
                    scalar1=mv[:, 0:1], scalar2=rstd, op0=OP.subtract, op1=OP.mult)
            # h_out = (hn*gn_g + gn_b + skip*xc) * silu(z)
            t1 = ph8.tile([P, CIN], F32, tag="ho_t1")
            nc.vector.tensor_mul(t1, hn, gn_g_bc)
            nc.vector.tensor_add(t1, t1, gn_b_bc)
            xct = ph8.tile([P, CIN], BF16, tag="xc_in", bufs=3)
            nc.sync.dma_start(out=xct, in_=xc_v[:, tt_i, :])
            t2 = ph8.tile([P, CIN], F32, tag="ho_t2")
            nc.vector.tensor_mul(t2, xct, skip_bc)
            nc.vector.tensor_add(t1, t1, t2)
            zt = ph8.tile([P, CIN], BF16, tag="z_in", bufs=3)
            nc.sync.dma_start(out=zt, in_=z_v[:, tt_i, :])
            sz = ph8.tile([P, CIN], BF16, tag="ho_sz")
            nc.scalar.activation(out=sz, in_=zt, func=AF.Silu)
            hout = ph8.tile([P, CIN], BF16, tag="hout")
            nc.vector.tensor_mul(hout, t1, sz)
            for c in range(CIN // P):
                pt = ps8.tile([P, P], BF16, tag="tr_ho", bufs=3)
                nc.tensor.transpose(pt, hout[:, ts(c, P)], ident)
                nc.any.tensor_copy(houtT[:, c, ts(tt_i, P)], pt)
        # down projection (partial over my channels) -> dram
        for tt_i in range(TT):
            pt = ps8.tile([P, EMB], F32, tag="down_ps", bufs=3)
            for c in range(CIN // P):
                nc.tensor.matmul(pt, lhsT=houtT[:, c, ts(tt_i, P)], rhs=w_down[:, c, :],
                                 start=(c == 0), stop=(c == CIN // P - 1))
            pb = ph8.tile([P, EMB], BF16, tag="par_t", bufs=3)
            nc.any.tensor_copy(pb, pt)
            nc.sync.dma_start(out=par_v[:, tt_i, :], in_=pb)
        nc.gpsimd.collective_compute(
            "AllReduce", OP.add, replica_groups=RG, ins=[par_d.opt()], outs=[ar_d.opt()])
        # residual: x += allreduced partial (stream through sbuf)
        for tt_i in range(TT):
            art = ph8.tile([P, EMB], BF16, tag="ar_t", bufs=3)
            nc.sync.dma_start(out=art, in_=ar_v[:, tt_i, :])
            xt = ph8.tile([P, EMB], F32, tag="x_t", bufs=3)
            nc.sync.dma_start(out=xt, in_=xdr[:, tt_i, :])
            xo = ph8.tile([P, EMB], F32, tag="xo_t", bufs=3)
            nc.vector.tensor_add(xo, xt, art)
            nc.sync.dma_start(out=xdr[:, tt_i, :], in_=xo)

    if dbg is not None:
        with tc.tile_pool(name=f"dbgx{kb}", bufs=2) as dp:
            for tt_i in range(TT):
                xt = dp.tile([P, EMB], F32, tag="dbg_x")
                nc.sync.dma_start(out=xt, in_=xdr[:, tt_i, :])
                nc.sync.dma_start(
                    out=dbg[f"x_after{kb}"][:].rearrange("(n p) c -> p n c", p=P)[:, tt_i, :],
                    in_=xt)


def _phase56(nc, tc, pools, kb, rowq, cbq, zeroP, ecol, egcol, ekcol_f,
             ekcol_b, sfac_bc, S_st, z_st, h_v, qT, kT, v_tok, trimask,
             ones_col, ident, br_f, dbg):
    dram = pools["dram"]

    # ---- phase 5: decay rows (all at partition base 0, blocks b0/b1/b2
    # of rowq, aliased as quantities die) ----
    nc.vector.memset(zeroP[:, 0:1], 0.0)
    nc.vector.memset(zeroP[:, 1:2], -1.0)

    def zat(n):
        return zeroP[0:HPC, 0:1].to_broadcast((HPC, n))

    def negat(n):
        return zeroP[0:HPC, 1:2].to_broadcast((HPC, n))

    b0, b1, b2 = _qs(rowq, 0), _qs(rowq, 1), _qs(rowq, 2)
    # b0 = i_raw, b1 = f_raw (written in phase 4)
    # lf = log_sigmoid(f) = ln(sigmoid(f));  P = cumsum(-lf) via the scan
    # recurrence state' = (lf - state) * -1  =  state - lf
    lf = b2
    nc.scalar.activation(out=lf, in_=b1, func=AF.Sigmoid)
    nc.scalar.activation(out=lf, in_=lf, func=AF.Ln)
    Pr = b1                                   # f dead
    nc.vector.tensor_tensor_scan(out=Pr, data0=lf, data1=negat(S),
                                 initial=0.0, op0=OP.subtract, op1=OP.mult)
    g = b2                                    # sp dead
    nc.vector.tensor_add(g, b0, Pr)           # g = i + P   (i dead)
    M = b0
    nc.vector.tensor_tensor_scan(out=M, data0=g, data1=zat(S),
                                 initial=-1e30, op0=OP.max, op1=OP.add)
    enm = b1                                  # P dead after this sub
    nc.vector.tensor_sub(enm, Pr, M)          # in place over P
    nc.scalar.activation(out=enm, in_=enm, func=AF.Exp)

    g3 = g.rearrange("p (c l) -> p c l", l=L)
    M3 = M.rearrange("p (c l) -> p c l", l=L)
    cm, mi, me, sf = (_cs(cbq, C_CM), _cs(cbq, C_MI), _cs(cbq, C_ME),
                      _cs(cbq, C_SF))
    nc.vector.tensor_reduce(out=cm, in_=g3, axis=mybir.AxisListType.X, op=OP.max)
    nc.vector.tensor_tensor_scan(out=mi, data0=cm, data1=zat(NCH),
                                 initial=-1e30, op0=OP.max, op1=OP.add)
    nc.vector.tensor_copy(me[:, 0:1], cm[:, 0:1])
    nc.vector.tensor_copy(me[:, 1:NCH], mi[:, 0:NCH - 1])
    nc.vector.tensor_sub(sf, me, mi)
    nc.scalar.activation(out=sf, in_=sf, func=AF.Exp)

    me_b = me[:, :, None].to_broadcast((HPC, NCH, L))
    mi_b = mi[:, :, None].to_broadcast((HPC, NCH, L))
    br_sf = dram.tile([HPC, NCH], F32, name=f"br_sf{kb}")

    if dbg is not None:
        nc.sync.dma_start(out=dbg[f"g{kb}"][:], in_=g)
        nc.sync.dma_start(out=dbg[f"M{kb}"][:], in_=M)

    # er = exp(me - M): in place over M (last use of M), bounce, then the
    # block is reused for eg and ek in turn.
    er = b0
    e3 = er.rearrange("p (c l) -> p c l", l=L)
    nc.vector.tensor_tensor(out=e3, in0=M3, in1=me_b, op=OP.subtract)
    nc.scalar.activation(out=er, in_=er, func=AF.Exp, scale=-1.0)
    nc.sync.dma_start(out=br_f[0], in_=er)
    eg = b0
    eg3 = eg.rearrange("p (c l) -> p c l", l=L)
    nc.vector.tensor_tensor(out=eg3, in0=g3, in1=me_b, op=OP.subtract)
    nc.scalar.activation(out=eg, in_=eg, func=AF.Exp)
    nc.sync.dma_start(out=br_f[1], in_=eg)
    ek = b0
    ek3 = ek.rearrange("p (c l) -> p c l", l=L)
    nc.vector.tensor_tensor(out=ek3, in0=g3, in1=mi_b, op=OP.subtract)
    nc.scalar.activation(out=ek, in_=ek, func=AF.Exp)
    nc.sync.dma_start(out=br_f[2], in_=ek)
    nc.sync.dma_start(out=br_sf, in_=sf)
    for hh in range(HPC):
        nc.sync.dma_start(out=ecol[:, hh, :], in_=br_f[0, hh].rearrange("(c l) -> l c", l=L))
        nc.sync.dma_start(out=egcol[:, hh, :], in_=br_f[1, hh].rearrange("(c l) -> l c", l=L))
        nc.sync.dma_start(out=ekcol_f[:, hh, :], in_=br_f[2, hh].rearrange("(c l) -> l c", l=L))
        nc.sync.dma_start(out=sfac_bc[:, hh, :],
                          in_=br_sf[hh:hh + 1, :].to_broadcast((P, NCH)))
    nc.vector.tensor_copy(ekcol_b, ekcol_f)

    # ---- phase 6: chunk loop ----
    rs_d = dram.tile([HPC, S], F32, name=f"rs_d{kb}")
    nc.vector.memset(S_st, 0.0)
    nc.vector.memset(z_st, 0.0)
    with tc.tile_pool(name=f"ph6ps_{kb}", bufs=1, space="PSUM") as ps6, \
         tc.tile_pool(name=f"ph6_{kb}", bufs=2) as ph6:
        for ci in range(NCH):
            rs = ps6.tile([1, HPC * P], F32, tag="rs_ps", bufs=1)
            for hh in range(HPC):
                co0 = 2 * hh  # head's first 128-col tile in qT/kT
                qk = ps6.tile([P, P], F32, tag="qk_ps", bufs=2)
                for c in range(2):
                    nc.tensor.matmul(qk, lhsT=kT[:, co0 + c, ts(ci, P)],
                                     rhs=qT[:, co0 + c, ts(ci, P)],
                                     start=(c == 0), stop=(c == 1))
                Cb = ph6.tile([P, P], BF16, tag="C_bf")
                nc.vector.scalar_tensor_tensor(
                    out=Cb, in0=qk, scalar=egcol[:, hh, ci:ci + 1], in1=trimask,
                    op0=OP.mult, op1=OP.mult)
                # h accumulation: intra + inter
                hps = ps6.tile([P, DH], F32, tag="h_ps", bufs=2)
                nc.tensor.matmul(hps, lhsT=Cb, rhs=v_tok[:, ci, hh * DH:(hh + 1) * DH],
                                 start=True, stop=(ci == 0))
                if ci > 0:
                    for c in range(2):
                        nc.tensor.matmul(hps, lhsT=qT[:, co0 + c, ts(ci, P)],
                                         rhs=S_st[:, hh, c, :],
                                         start=False, stop=(c == 1))
                # row-sums for the normalizer (head hh in free range hh*128:)
                nc.tensor.matmul(rs[:, ts(hh, P)], lhsT=ones_col, rhs=Cb,
                                 start=True, stop=(ci == 0))
                if ci > 0:
                    for c in range(2):
                        nc.tensor.matmul(rs[:, ts(hh, P)], lhsT=z_st[:, hh, c:c + 1],
                                         rhs=qT[:, co0 + c, ts(ci, P)],
                                         start=False, stop=(c == 1))
                # h = ecol * (intra + inter) -> stream to DRAM
                hb = ph6.tile([P, DH], BF16, tag="h_out", bufs=3)
                nc.vector.tensor_scalar(
                    out=hb, in0=hps, scalar1=ecol[:, hh, ci:ci + 1],
                    scalar2=None, op0=OP.mult)
                nc.sync.dma_start(out=h_v[:, ci, hh * DH:(hh + 1) * DH], in_=hb)
                # state update
                ktp = ps6.tile([P, DH], BF16, tag="ktp_ps")
                for c in range(2):
                    nc.tensor.transpose(ktp[:, ts(c, P)], kT[:, co0 + c, ts(ci, P)],
                                        ident)
                ktok = ph6.tile([P, DH], BF16, tag="ktok")
                nc.any.tensor_copy(ktok, ktp)
                vtil = ph6.tile([P, DH], BF16, tag="vtil")
                nc.vector.tensor_scalar(
                    out=vtil, in0=v_tok[:, ci, hh * DH:(hh + 1) * DH],
                    scalar1=ekcol_f[:, hh, ci:ci + 1], scalar2=None, op0=OP.mult)
                Sd = ps6.tile([P, 2, DH], F32, tag="Sd_ps")
                for c in range(2):
                    nc.tensor.matmul(Sd[:, c, :], lhsT=ktok[:, ts(c, P)], rhs=vtil,
                                     start=True, stop=True)
                zd = ps6.tile([P, 2], F32, tag="zd_ps")
                for c in range(2):
                    nc.tensor.matmul(zd[:, c:c + 1], lhsT=ktok[:, ts(c, P)],
                                     rhs=ekcol_b[:, hh, ci:ci + 1],
                                     start=True, stop=True)
                nc.vector.tensor_scalar(
                    out=S_st[:, hh], in0=S_st[:, hh],
                    scalar1=sfac_bc[:, hh, ci:ci + 1], scalar2=None, op0=OP.mult)
                nc.vector.tensor_add(S_st[:, hh], Sd, S_st[:, hh])
                nc.vector.tensor_scalar(
                    out=z_st[:, hh], in0=z_st[:, hh],
                    scalar1=sfac_bc[:, hh, ci:ci + 1], scalar2=None, op0=OP.mult)
                nc.vector.tensor_add(z_st[:, hh], zd, z_st[:, hh])
            rsp = ph6.tile([1, HPC * P], F32, tag="rs_sb", bufs=3)
            nc.scalar.activation(out=rsp, in_=rs, func=AF.Copy)
            for hh in range(HPC):
                nc.sync.dma_start(out=rs_d[hh, ts(ci, P)], in_=rsp[:, ts(hh, P)])
    return rs_d


def build(debug=False):
    nc = bacc.Bacc(num_devices=NCORES)
    t = _decl_inputs(nc)
    out = nc.dram_tensor("out", [S, EMB], F32, kind="ExternalOutput")
    dbg = None
    if debug:
        dbg = {}
        for kb in (0, 1):
            for nm in ("g", "M", "er", "enm", "rn"):
                dbg[f"{nm}{kb}"] = nc.dram_tensor(f"d_{nm}{kb}", [HPC, S], F32, kind="ExternalOutput")
            dbg[f"h{kb}"] = nc.dram_tensor(f"d_h{kb}", [S, CIN], BF16, kind="ExternalOutput")
            dbg[f"x_after{kb}"] = nc.dram_tensor(f"d_x_after{kb}", [S, EMB], F32, kind="ExternalOutput")

    with tile.TileContext(nc) as tc, ExitStack() as ctx:
        consts = ctx.enter_context(tc.tile_pool(name="consts", bufs=1))
        small = ctx.enter_context(tc.tile_pool(name="small", bufs=4))
        dram = ctx.enter_context(tc.tile_pool(name="dram", bufs=1, space="DRAM"))
        ident = consts.tile([P, P], BF16, name="ident")
        make_identity(nc, ident)
        trimask = consts.tile([P, P], F32, name="trimask")
        make_upper_triangular(nc, trimask, val=1.0, diag=True)
        ones_col = consts.tile([P, 1], BF16, name="ones_col")
        nc.vector.memset(ones_col, 1.0)
        eps_t = consts.tile([P, 1], F32, name="eps_t")
        nc.vector.memset(eps_t, EPS)
        pools = {"consts": consts, "small": small, "dram": dram, "ident": ident,
                 "trimask": trimask, "ones_col": ones_col, "eps": eps_t}

        x_d = dram.tile([S, EMB], F32, name="x_master")
        xdr = x_d[:].rearrange("(n p) c -> p n c", p=P)

        # stage 1: x = x1 @ w_init + x2' -> x_d   (b_init folded into x2
        # on the host)
        with tc.tile_pool(name="s1", bufs=1) as s1p, \
             tc.tile_pool(name="s1ps", bufs=3, space="PSUM") as s1ps:
            w_init_sb = _load_w(nc, s1p, t["w_init"], "w_init_sb")  # [128,2,512]
            x1T_sb = s1p.tile([P, INIT // P, S], BF16, name="x1T_sb")
            nc.sync.dma_start(out=x1T_sb, in_=t["x1T"][:].rearrange("(ko p) n -> p ko n", p=P))
            x2_sb = s1p.tile([P, TT, EMB], F32, name="x2_sb")
            nc.sync.dma_start(out=x2_sb, in_=t["x2"][:].rearrange("(n p) c -> p n c", p=P))
            for tt_i in range(TT):
                pt = s1ps.tile([P, EMB], F32, tag="s1_ps")
                for k in range(INIT // P):
                    nc.tensor.matmul(pt, lhsT=x1T_sb[:, k, ts(tt_i, P)],
                                     rhs=w_init_sb[:, k, :],
                                     start=(k == 0), stop=(k == INIT // P - 1))
                xo = s1p.tile([P, EMB], F32, tag="s1_out", bufs=3)
                nc.vector.tensor_add(xo, pt, x2_sb[:, tt_i, :])
                nc.sync.dma_start(out=xdr[:, tt_i, :], in_=xo)

        for kb in (0, 1):
            with ExitStack() as bctx:
                _mlstm_block(nc, bctx, tc, pools, t, kb, xdr, dbg)

        # final LN -> out
        with tc.tile_pool(name="fin", bufs=2) as fp:
            g_bc = _bcast(nc, fp, pools, t["ln_out_g"], "lno_g_bc")
            b_bc = _bcast(nc, fp, pools, t["ln_out_b"], "lno_b_bc")
            out_v = out[:].rearrange("(n p) c -> p n c", p=P)
            for tt_i in range(TT):
                xt = small.tile([P, EMB], F32, tag="ln_x", bufs=3)
                nc.sync.dma_start(out=xt, in_=xdr[:, tt_i, :])
                ot = fp.tile([P, EMB], F32, tag="out_t", bufs=3)
                _ln_tile(nc, pools, xt, g_bc, b_bc, ot)
                nc.sync.dma_start(out=out_v[:, tt_i, :], in_=ot)
    nc.finalize()
    return nc


def _prep_inputs_for_core(core, x1, x2, params):
    b, g = core // 2, core % 2
    myc = np.arange(g * CIN, (g + 1) * CIN)
    otc = np.arange((1 - g) * CIN, (2 - g) * CIN)
    perm = np.concatenate([myc, otc])
    hs = [2 * g, 2 * g + 1]

    def bf(a):
        return np.ascontiguousarray(np.asarray(a, np.float32).astype(ml_dtypes.bfloat16))

    def f32(a):
        return np.ascontiguousarray(np.asarray(a, np.float32))

    m = {
        "x1T": bf(np.asarray(x1[b]).T),
        "x2": f32(np.asarray(x2[b], np.float32) + np.asarray(params["b_init"], np.float32)[None, :]),
        "w_init": bf(params["W_init"]),
        "ln_out_g": f32(params["ln_out_g"]).reshape(1, EMB),
        "ln_out_b": f32(params["ln_out_b"]).reshape(1, EMB),
    }
    for kb, blk in enumerate(params["blocks"]):
        sf = f"_{kb}"
        W_up = np.asarray(blk["W_up"], np.float32)
        w_up_x = W_up[:, :INNER][:, perm]
        w_up_z = W_up[:, INNER:][:, myc]
        m["w_up" + sf] = bf(np.concatenate([w_up_x, w_up_z], axis=1))
        Wv = np.asarray(blk["Wv"], np.float32)[:, myc]
        m["wv" + sf] = bf(W_up[:, :INNER] @ Wv)   # folded: v = xn @ (W_up_xin @ Wv)
        m["ln_g" + sf] = f32(blk["ln_g"]).reshape(1, EMB)
        m["ln_b" + sf] = f32(blk["ln_b"]).reshape(1, EMB)
        cw = np.asarray(blk["conv_w"], np.float32)  # [K, INNER]
        m["conv_w" + sf] = f32(cw.T[perm])          # [INNER, K]
        m["conv_b" + sf] = f32(np.asarray(blk["conv_b"], np.float32)[perm])
        m["wq" + sf] = bf(np.asarray(blk["Wq"], np.float32)[perm][:, myc])
        m["wk" + sf] = bf(np.asarray(blk["Wk"], np.float32)[perm][:, myc])
        Wi = np.asarray(blk["Wi"], np.float32)[perm][:, hs]
        Wf = np.asarray(blk["Wf"], np.float32)[perm][:, hs]
        m["wif" + sf] = bf(np.concatenate([Wi, Wf], axis=1))
        bi = np.asarray(blk["bi"], np.float32)[hs]
        bfr = np.asarray(blk["bf"], np.float32)[hs]
        m["bif" + sf] = f32(np.concatenate([bi, bfr])).reshape(2 * HPC, 1)
        m["skip" + sf] = f32(np.asarray(blk["skip"], np.float32)[myc]).reshape(1, CIN)
        m["gn_g" + sf] = f32(np.asarray(blk["gn_g"], np.float32)[myc]).reshape(1, CIN)
        m["gn_b" + sf] = f32(np.asarray(blk["gn_b"], np.float32)[myc]).reshape(1, CIN)
        m["w_down" + sf] = bf(np.asarray(blk["W_down"], np.float32)[myc, :])
    return m


def _get_nc(debug=False):
    key = "dbg" if debug else "main"
    if key not in _cached:
        _cached[key] = build(debug=debug)
    return _cached[key]


def run(x1, x2, params, debug=False, trace=False):
    nc = _get_nc(debug=debug)
    x1 = np.asarray(x1, np.float32)
    x2 = np.asarray(x2, np.float32)
    in_maps = [_prep_inputs_for_core(c, x1, x2, params) for c in range(NCORES)]
    res = run_bass_kernel_spmd(nc, in_maps, core_ids=list(range(NCORES)), trace=trace)
    out = np.stack([res.results[2 * b]["out"] for b in range(B)], axis=0)
    return out, res


def kernel(x1, x2, params):
    out, _ = run(x1, x2, params)
    return out
